# revision 7
# baseline (speedup 1.0000x reference)
"""Trainium2 Bass kernel for nn_BCNet: three-way low-rank bilinear net.

reference:
  v_ = relu(v @ Wv.T + bv)            # (B, NV, HK)
  q_ = relu(q @ Wq.T + bq)            # (B, NQ, HK)
  logits = einsum('hk,bvk,bqk->bhvq', h_mat, v_, q_) + h_bias

Sharding: data-parallel over batch, 4 batch items per core (8 cores).
All matmuls in bf16 with fp32 PSUM accumulation.

v2 schedule (from trace analysis of the 299.9us baseline):
  - PE is at peak cadence (215.7ns per 512-free matmul, 1248 matmuls =
    269.5us); all slack is head (12.4us to first matmul + stage-B DMA
    stalls) and tail (serialized adds/stores after last matmul).
  - Warmup: a few junk matmuls right after the preamble ride out the
    PE p-state ramp while the first DMAs land.
  - Stages A/B run as two 6-wide PSUM passes (6 banks) so the first
    pass consumes one (weight-half, rhs) chunk per 6 matmuls
    (~250 GB/s < the ~390 GB/s single-queue wire rate): no DMA stalls
    and the PE can start as soon as the first half-chunks land.
  - All input DMAs are issued on Sync in exact consumption order so
    the single queue delivers chunks just in time.
  - Stage C is head-major: psum [128q x 512v] per (b,h); h_bias[h] is
    a per-partition constant so the Scalar activation (Identity,
    bias) does psum->SBUF + bias in one op; no DVE adds, one store
    per (b,h). Host transposes (q,v)->(v,q) at the end.
  - Chain ends are staggered (last two d's j-major) so the relu
    activations that free PSUM banks / produce vact fire early enough
    to never stall the next pass / stage C.

Host prep per core:
  vT   (4, 2048, 512) bf16  : v[b].T per batch item
  qT   (1024, 512)    bf16  : q[4c:4c+4] transposed+stacked, cols = b*128+q
  WvT  (2048, 1536)   bf16
  WqT  (1024, 1536)   bf16
  bvT  (128, 12) f32 : bv[jc*128+p]
  bqT  (128, 12) f32
  hm   (128, 12, 8) f32 : h_mat[h, jc*128+p]
  hb   (128, 8)  f32 : h_bias[h] broadcast over partitions
Device output per core: out (4, 8, 128, 512) f32 = [b, h, q, v].
Host post: concat -> (32, 8, 128, 512) -> transpose -> (32, 8, 512, 128).
"""

import numpy as np

B, NV, NQ = 32, 512, 128
V_DIM, Q_DIM, HK, H_OUT = 2048, 1024, 1536, 8
N_CORES = 8
BPC = B // N_CORES          # 4 batch items per core
JC = HK // 128              # 12 k-chunks
DCV = V_DIM // 128          # 16 contraction chunks for v
DCQ = Q_DIM // 128          # 8 contraction chunks for q

_CACHE = {}


def _build_nc():
    import concourse.tile as tile
    from concourse import bacc, mybir
    from contextlib import ExitStack

    bf16 = mybir.dt.bfloat16
    f32 = mybir.dt.float32

    nc = bacc.Bacc()

    vT = nc.declare_dram_parameter("vT", [BPC, V_DIM, NV], bf16, isOutput=False)
    qT = nc.declare_dram_parameter("qT", [Q_DIM, BPC * NQ], bf16, isOutput=False)
    WvT = nc.declare_dram_parameter("WvT", [V_DIM, HK], bf16, isOutput=False)
    WqT = nc.declare_dram_parameter("WqT", [Q_DIM, HK], bf16, isOutput=False)
    bvT = nc.declare_dram_parameter("bvT", [128, JC], f32, isOutput=False)
    bqT = nc.declare_dram_parameter("bqT", [128, JC], f32, isOutput=False)
    hm = nc.declare_dram_parameter("hm", [128, JC, H_OUT], f32, isOutput=False)
    hb = nc.declare_dram_parameter("hb", [128, H_OUT], f32, isOutput=False)
    out = nc.declare_dram_parameter("out", [BPC, H_OUT, NQ, NV], f32, isOutput=True)

    with ExitStack() as ctx:
        tc = ctx.enter_context(tile.TileContext(nc))
        consts = ctx.enter_context(tc.tile_pool(name="consts", bufs=1))
        qpool = ctx.enter_context(tc.tile_pool(name="qpool", bufs=1))
        vin = ctx.enter_context(tc.tile_pool(name="vin", bufs=2))
        vact = ctx.enter_context(tc.tile_pool(name="vact", bufs=2))
        qhp = ctx.enter_context(tc.tile_pool(name="qhp", bufs=1))
        junkp = ctx.enter_context(tc.tile_pool(name="junkp", bufs=1))
        outp = ctx.enter_context(tc.tile_pool(name="outp", bufs=3))
        psAB = ctx.enter_context(tc.tile_pool(name="psAB", bufs=6, space="PSUM"))
        psC = ctx.enter_context(tc.tile_pool(name="psC", bufs=2, space="PSUM"))

        # ---- SBUF destination tiles ----
        qt_sb = qpool.tile([128, DCQ, BPC * NQ], bf16)
        wq_sb = consts.tile([128, DCQ, HK], bf16)
        wv_sb = consts.tile([128, DCV, HK], bf16)
        bq_sb = consts.tile([128, JC], f32)
        bv_sb = consts.tile([128, JC], f32)
        hm_sb = consts.tile([128, JC, H_OUT], f32)
        hb_sb = consts.tile([128, H_OUT], f32)
        vt0_sb = vin.tile([128, DCV, NV], bf16, tag="vt", name="vt0")

        qT_r = qT.rearrange("(d p) n -> p d n", p=128)
        WqT_r = WqT.rearrange("(d p) j -> p d j", p=128)
        WvT_r = WvT.rearrange("(d p) j -> p d j", p=128)
        vT0_r = vT[0].rearrange("(d p) n -> p d n", p=128)

        # ---- warmup: junk matmuls to ride out the PE p-state ramp while
        # the first DMAs land.  The junk tile is memset on the Tensor engine
        # itself so no cross-engine dependency delays the start.
        junk = junkp.tile([128, NV], bf16)
        nc.vector.memset(junk, 0.0)
        ps_junk = psAB.tile([128, NV], f32, tag="psAB", name="ps_junk")
        for w in range(8):
            nc.tensor.matmul(
                ps_junk, lhsT=junk[:, 0:128], rhs=junk,
                start=(w == 0), stop=(w == 7),
            )

        # ---- input DMAs: tiny consts on GpSimd; stage-B weight chunks on
        # Vector racing the qt chunks on Sync (two queues through the DMA
        # bandwidth ramp-up); everything in exact consumption order.
        nc.gpsimd.dma_start(out=bq_sb, in_=bqT[:, :])
        nc.gpsimd.dma_start(out=hm_sb, in_=hm[:, :, :])
        nc.gpsimd.dma_start(out=bv_sb, in_=bvT[:, :])
        nc.gpsimd.dma_start(out=hb_sb, in_=hb[:, :])

        # stage B pass1 inputs: qt chunks on Sync racing wq chunks on Scalar
        # (Scalar is idle until the first activations at ~18us)
        for d in range(DCQ):
            nc.sync.dma_start(out=qt_sb[:, d, :], in_=qT_r[:, d, :])
            nc.scalar.dma_start(out=wq_sb[:, d, 0:768], in_=WqT_r[:, d, 0:768])
        # stage B pass2 inputs
        for d in range(DCQ):
            nc.gpsimd.dma_start(out=wq_sb[:, d, 768:HK], in_=WqT_r[:, d, 768:HK])
        # stage A(b=0) pass1 inputs: vt0 groups interleaved with wv halves
        for g in range(4):
            nc.sync.dma_start(out=vt0_sb[:, 4 * g:4 * g + 4, :],
                              in_=vT0_r[:, 4 * g:4 * g + 4, :])
            for d in range(4 * g, 4 * g + 4):
                nc.sync.dma_start(out=wv_sb[:, d, 0:768], in_=WvT_r[:, d, 0:768])
        # stage A pass2 inputs
        for d in range(DCV):
            nc.gpsimd.dma_start(out=wv_sb[:, d, 768:HK], in_=WvT_r[:, d, 768:HK])

        def mm_pass(ps_pool, w_sb, x_sb, act_sb, b_sb, j0, nj, ndc, tagp):
            """One 6-wide pass: chains j0..j0+nj-1 accumulate over ndc
            d-chunks, d-major except the last two d's which run j-major so
            chain ends (and their activations) stagger early."""
            pss = [ps_pool.tile([128, NV], f32, tag="psAB", name=f"{tagp}_{j0 + i}")
                   for i in range(nj)]
            for d in range(ndc - 2):
                for i in range(nj):
                    j = j0 + i
                    nc.tensor.matmul(
                        pss[i],
                        lhsT=w_sb[:, d, j * 128:(j + 1) * 128],
                        rhs=x_sb[:, d, :],
                        start=(d == 0),
                        stop=False,
                    )
            for i in range(nj):
                j = j0 + i
                for d in (ndc - 2, ndc - 1):
                    nc.tensor.matmul(
                        pss[i],
                        lhsT=w_sb[:, d, j * 128:(j + 1) * 128],
                        rhs=x_sb[:, d, :],
                        start=False,
                        stop=(d == ndc - 1),
                    )
                nc.scalar.activation(
                    out=act_sb[:, j, :],
                    in_=pss[i],
                    func=mybir.ActivationFunctionType.Relu,
                    bias=b_sb[:, j:j + 1],
                    scale=1.0,
                )

        # ---- stage B: q_ = relu(q @ Wq.T + bq), all 4 b at once ----
        qact_sb = qpool.tile([128, JC, BPC * NQ], bf16)
        mm_pass(psAB, wq_sb, qt_sb, qact_sb, bq_sb, 0, 6, DCQ, "psB1")
        mm_pass(psAB, wq_sb, qt_sb, qact_sb, bq_sb, 6, 6, DCQ, "psB2")

        for b in range(BPC):
            # ---- stage A: v_[b] = relu(v[b] @ Wv.T + bv), transposed layout
            if b == 0:
                vt_sb = vt0_sb
            else:
                vt_sb = vin.tile([128, DCV, NV], bf16, tag="vt")
                vT_r = vT[b].rearrange("(d p) n -> p d n", p=128)
                nc.sync.dma_start(out=vt_sb[:, :, :], in_=vT_r[:, :, :])
            vact_sb = vact.tile([128, JC, NV], bf16, tag="vact")
            mm_pass(psAB, wv_sb, vt_sb, vact_sb, bv_sb, 0, 6, DCV, f"psA1_{b}")
            mm_pass(psAB, wv_sb, vt_sb, vact_sb, bv_sb, 6, 6, DCV, f"psA2_{b}")

            # ---- build Qh[b][k, h*128+q'] = q_[k, b*128+q'] * h_mat[h, k]
            qh_sb = qhp.tile([128, JC, H_OUT * NQ], bf16, tag="qh")
            for j in range(JC):
                for h in range(H_OUT):
                    nc.vector.tensor_scalar_mul(
                        qh_sb[:, j, h * NQ:(h + 1) * NQ],
                        qact_sb[:, j, b * NQ:(b + 1) * NQ],
                        hm_sb[:, j, h:h + 1],
                    )

            # ---- stage C: logits[b,h] = (qh_h).T @ v_  -> psum [128q, 512v]
            for h in range(H_OUT):
                po = psC.tile([128, NV], f32, tag="psC")
                for j in range(JC):
                    nc.tensor.matmul(
                        po,
                        lhsT=qh_sb[:, j, h * NQ:(h + 1) * NQ],
                        rhs=vact_sb[:, j, :],
                        start=(j == 0),
                        stop=(j == JC - 1),
                    )
                o_sb = outp.tile([128, NV], f32, tag="osb")
                last = (b == BPC - 1 and h == H_OUT - 1)
                if not last:
                    nc.scalar.activation(
                        out=o_sb,
                        in_=po,
                        func=mybir.ActivationFunctionType.Identity,
                        bias=hb_sb[:, h:h + 1],
                        scale=1.0,
                    )
                    eng = nc.gpsimd if h % 2 == 0 else nc.sync
                    eng.dma_start(out=out[b, h, :, :], in_=o_sb)
                else:
                    # split the very last store so its activation, issue and
                    # transfer pipeline instead of serializing after the
                    # final matmul
                    for half, eng in ((0, nc.gpsimd), (1, nc.sync)):
                        sl = slice(half * 256, (half + 1) * 256)
                        nc.scalar.activation(
                            out=o_sb[:, sl],
                            in_=po[:, sl],
                            func=mybir.ActivationFunctionType.Identity,
                            bias=hb_sb[:, h:h + 1],
                            scale=1.0,
                        )
                        eng.dma_start(out=out[b, h, :, sl], in_=o_sb[:, sl])

    nc.compile()
    return nc


def kernel(v, q, Wv, bv, Wq, bq, h_mat, h_bias):
    import ml_dtypes
    from concourse import bass_utils

    bf16 = ml_dtypes.bfloat16

    if "nc" not in _CACHE:
        _CACHE["nc"] = _build_nc()
    nc = _CACHE["nc"]

    v = np.asarray(v, dtype=np.float32)
    q = np.asarray(q, dtype=np.float32)
    Wv = np.asarray(Wv, dtype=np.float32)
    Wq = np.asarray(Wq, dtype=np.float32)
    bv = np.asarray(bv, dtype=np.float32)
    bq = np.asarray(bq, dtype=np.float32)
    h_mat = np.asarray(h_mat, dtype=np.float32)
    h_bias = np.asarray(h_bias, dtype=np.float32)

    vT = np.ascontiguousarray(v.transpose(0, 2, 1)).astype(bf16)      # (B, 2048, 512)
    WvT = np.ascontiguousarray(Wv.T).astype(bf16)                     # (2048, 1536)
    WqT = np.ascontiguousarray(Wq.T).astype(bf16)                     # (1024, 1536)
    bvT = np.ascontiguousarray(bv.reshape(JC, 128).T)                 # (128, 12)
    bqT = np.ascontiguousarray(bq.reshape(JC, 128).T)
    # hm[p, jc, h] = h_mat[h, jc*128+p]
    hmP = np.ascontiguousarray(h_mat.reshape(H_OUT, JC, 128).transpose(2, 1, 0))
    hbB = np.ascontiguousarray(np.broadcast_to(h_bias[None, :], (128, H_OUT)))

    in_maps = []
    for c in range(N_CORES):
        bs = slice(BPC * c, BPC * (c + 1))
        qTc = np.ascontiguousarray(
            q[bs].transpose(2, 0, 1).reshape(Q_DIM, BPC * NQ)
        ).astype(bf16)
        in_maps.append({
            "vT": vT[bs],
            "qT": qTc,
            "WvT": WvT,
            "WqT": WqT,
            "bvT": bvT,
            "bqT": bqT,
            "hm": hmP,
            "hb": hbB,
        })

    res = bass_utils.run_bass_kernel_spmd(nc, in_maps, list(range(N_CORES)))
    outs = np.concatenate([res.results[c]["out"] for c in range(N_CORES)], axis=0)
    # (32, 8, 128, 512) -> (32, 8, 512, 128)
    logits = outs.transpose(0, 1, 3, 2)
    return np.ascontiguousarray(logits)


# revision 9
# speedup vs baseline: 1.0248x; 1.0248x over previous
"""Trainium2 Bass kernel for nn_BCNet: three-way low-rank bilinear net.

reference:
  v_ = relu(v @ Wv.T + bv)            # (B, NV, HK)
  q_ = relu(q @ Wq.T + bq)            # (B, NQ, HK)
  logits = einsum('hk,bvk,bqk->bhvq', h_mat, v_, q_) + h_bias

Sharding: data-parallel over batch, 4 batch items per core (8 cores).
All matmuls in bf16 with fp32 PSUM accumulation.

v2 schedule (from trace analysis of the 299.9us baseline):
  - PE is at peak cadence (215.7ns per 512-free matmul, 1248 matmuls =
    269.5us); all slack is head (12.4us to first matmul + stage-B DMA
    stalls) and tail (serialized adds/stores after last matmul).
  - Warmup: a few junk matmuls right after the preamble ride out the
    PE p-state ramp while the first DMAs land.
  - Stages A/B run as two 6-wide PSUM passes (6 banks) so the first
    pass consumes one (weight-half, rhs) chunk per 6 matmuls
    (~250 GB/s < the ~390 GB/s single-queue wire rate): no DMA stalls
    and the PE can start as soon as the first half-chunks land.
  - All input DMAs are issued on Sync in exact consumption order so
    the single queue delivers chunks just in time.
  - Stage C is head-major: psum [128q x 512v] per (b,h); h_bias[h] is
    a per-partition constant so the Scalar activation (Identity,
    bias) does psum->SBUF + bias in one op; no DVE adds, one store
    per (b,h). Host transposes (q,v)->(v,q) at the end.
  - Chain ends are staggered (last two d's j-major) so the relu
    activations that free PSUM banks / produce vact fire early enough
    to never stall the next pass / stage C.

Host prep per core:
  vT   (4, 2048, 512) bf16  : v[b].T per batch item
  qT   (1024, 512)    bf16  : q[4c:4c+4] transposed+stacked, cols = b*128+q
  WvT  (2048, 1536)   bf16
  WqT  (1024, 1536)   bf16
  bvT  (128, 12) f32 : bv[jc*128+p]
  bqT  (128, 12) f32
  hm   (128, 12, 8) f32 : h_mat[h, jc*128+p]
  hb   (128, 8)  f32 : h_bias[h] broadcast over partitions
Device output per core: out (4, 8, 128, 512) f32 = [b, h, q, v].
Host post: concat -> (32, 8, 128, 512) -> transpose -> (32, 8, 512, 128).
"""

import numpy as np

B, NV, NQ = 32, 512, 128
V_DIM, Q_DIM, HK, H_OUT = 2048, 1024, 1536, 8
N_CORES = 8
BPC = B // N_CORES          # 4 batch items per core
JC = HK // 128              # 12 k-chunks
DCV = V_DIM // 128          # 16 contraction chunks for v
DCQ = Q_DIM // 128          # 8 contraction chunks for q

_CACHE = {}


def _build_nc():
    import concourse.tile as tile
    from concourse import bacc, mybir
    from contextlib import ExitStack

    bf16 = mybir.dt.bfloat16
    f32 = mybir.dt.float32

    nc = bacc.Bacc()

    vT = nc.declare_dram_parameter("vT", [BPC, V_DIM, NV], bf16, isOutput=False)
    qT = nc.declare_dram_parameter("qT", [Q_DIM, BPC * NQ], bf16, isOutput=False)
    WvT = nc.declare_dram_parameter("WvT", [V_DIM, HK], bf16, isOutput=False)
    WqT = nc.declare_dram_parameter("WqT", [Q_DIM, HK], bf16, isOutput=False)
    bvT = nc.declare_dram_parameter("bvT", [128, JC], f32, isOutput=False)
    bqT = nc.declare_dram_parameter("bqT", [128, JC], f32, isOutput=False)
    hm = nc.declare_dram_parameter("hm", [128, JC, H_OUT], f32, isOutput=False)
    hb = nc.declare_dram_parameter("hb", [128, H_OUT], f32, isOutput=False)
    out = nc.declare_dram_parameter("out", [BPC, H_OUT, NQ, NV], f32, isOutput=True)

    with ExitStack() as ctx:
        tc = ctx.enter_context(tile.TileContext(nc))
        consts = ctx.enter_context(tc.tile_pool(name="consts", bufs=1))
        qpool = ctx.enter_context(tc.tile_pool(name="qpool", bufs=1))
        vin = ctx.enter_context(tc.tile_pool(name="vin", bufs=2))
        vact = ctx.enter_context(tc.tile_pool(name="vact", bufs=2))
        qhp = ctx.enter_context(tc.tile_pool(name="qhp", bufs=1))
        junkp = ctx.enter_context(tc.tile_pool(name="junkp", bufs=1))
        outp = ctx.enter_context(tc.tile_pool(name="outp", bufs=3))
        psAB = ctx.enter_context(tc.tile_pool(name="psAB", bufs=6, space="PSUM"))
        psC = ctx.enter_context(tc.tile_pool(name="psC", bufs=2, space="PSUM"))

        # ---- SBUF destination tiles ----
        qt_sb = qpool.tile([128, DCQ, BPC * NQ], bf16)
        wq_sb = consts.tile([128, DCQ, HK], bf16)
        wv_sb = consts.tile([128, DCV, HK], bf16)
        bq_sb = consts.tile([128, JC], f32)
        bv_sb = consts.tile([128, JC], f32)
        hm_sb = consts.tile([128, JC, H_OUT], f32)
        hb_sb = consts.tile([128, H_OUT], f32)
        vt0_sb = vin.tile([128, DCV, NV], bf16, tag="vt", name="vt0")

        qT_r = qT.rearrange("(d p) n -> p d n", p=128)
        WqT_r = WqT.rearrange("(d p) j -> p d j", p=128)
        WvT_r = WvT.rearrange("(d p) j -> p d j", p=128)
        vT0_r = vT[0].rearrange("(d p) n -> p d n", p=128)

        # ---- warmup: junk matmuls to ride out the PE p-state ramp while
        # the first DMAs land.  The junk tile is memset on the Tensor engine
        # itself so no cross-engine dependency delays the start.
        junk = junkp.tile([128, NV], bf16)
        nc.vector.memset(junk, 0.0)
        ps_junk = psAB.tile([128, NV], f32, tag="psAB", name="ps_junk")
        for w in range(4):
            nc.tensor.matmul(
                ps_junk, lhsT=junk[:, 0:128], rhs=junk,
                start=(w == 0), stop=(w == 3),
            )

        # ---- input DMAs: tiny consts on GpSimd; stage-B weight chunks on
        # Vector racing the qt chunks on Sync (two queues through the DMA
        # bandwidth ramp-up); everything in exact consumption order.
        nc.gpsimd.dma_start(out=bq_sb, in_=bqT[:, :])
        nc.gpsimd.dma_start(out=hm_sb, in_=hm[:, :, :])
        nc.gpsimd.dma_start(out=bv_sb, in_=bvT[:, :])
        nc.gpsimd.dma_start(out=hb_sb, in_=hb[:, :])

        # bulk loads: ALL on the Sync queue (a single queue ramps to the full
        # ~390 GB/s wire rate; splitting across queues makes each ramp
        # slowly), interleaved in exact consumption order so the first chain
        # only waits for 327KB.
        for d in range(DCQ):
            nc.sync.dma_start(out=qt_sb[:, d, :], in_=qT_r[:, d, :])
            nc.sync.dma_start(out=wq_sb[:, d, 0:768], in_=WqT_r[:, d, 0:768])
        # stage B pass2 inputs
        for d in range(DCQ):
            nc.sync.dma_start(out=wq_sb[:, d, 768:HK], in_=WqT_r[:, d, 768:HK])
        # stage A(b=0) pass1 inputs: vt0 groups interleaved with wv halves
        for g in range(4):
            nc.sync.dma_start(out=vt0_sb[:, 4 * g:4 * g + 4, :],
                              in_=vT0_r[:, 4 * g:4 * g + 4, :])
            for d in range(4 * g, 4 * g + 4):
                nc.sync.dma_start(out=wv_sb[:, d, 0:768], in_=WvT_r[:, d, 0:768])
        # stage A pass2 inputs
        for d in range(DCV):
            nc.sync.dma_start(out=wv_sb[:, d, 768:HK], in_=WvT_r[:, d, 768:HK])

        def mm_pass(ps_pool, w_sb, x_sb, act_sb, b_sb, j0, nj, ndc, tagp):
            """One 6-wide pass: chains j0..j0+nj-1 accumulate over ndc
            d-chunks, d-major except the last two d's which run j-major so
            chain ends (and their activations) stagger early."""
            pss = [ps_pool.tile([128, NV], f32, tag="psAB", name=f"{tagp}_{j0 + i}")
                   for i in range(nj)]
            for d in range(ndc - 2):
                for i in range(nj):
                    j = j0 + i
                    nc.tensor.matmul(
                        pss[i],
                        lhsT=w_sb[:, d, j * 128:(j + 1) * 128],
                        rhs=x_sb[:, d, :],
                        start=(d == 0),
                        stop=False,
                    )
            for i in range(nj):
                j = j0 + i
                for d in (ndc - 2, ndc - 1):
                    nc.tensor.matmul(
                        pss[i],
                        lhsT=w_sb[:, d, j * 128:(j + 1) * 128],
                        rhs=x_sb[:, d, :],
                        start=False,
                        stop=(d == ndc - 1),
                    )
                nc.scalar.activation(
                    out=act_sb[:, j, :],
                    in_=pss[i],
                    func=mybir.ActivationFunctionType.Relu,
                    bias=b_sb[:, j:j + 1],
                    scale=1.0,
                )

        # ---- stage B: q_ = relu(q @ Wq.T + bq), all 4 b at once ----
        qact_sb = qpool.tile([128, JC, BPC * NQ], bf16)
        mm_pass(psAB, wq_sb, qt_sb, qact_sb, bq_sb, 0, 6, DCQ, "psB1")
        mm_pass(psAB, wq_sb, qt_sb, qact_sb, bq_sb, 6, 6, DCQ, "psB2")

        for b in range(BPC):
            # ---- stage A: v_[b] = relu(v[b] @ Wv.T + bv), transposed layout
            if b == 0:
                vt_sb = vt0_sb
            else:
                vt_sb = vin.tile([128, DCV, NV], bf16, tag="vt")
                vT_r = vT[b].rearrange("(d p) n -> p d n", p=128)
                nc.sync.dma_start(out=vt_sb[:, :, :], in_=vT_r[:, :, :])
            vact_sb = vact.tile([128, JC, NV], bf16, tag="vact")
            mm_pass(psAB, wv_sb, vt_sb, vact_sb, bv_sb, 0, 6, DCV, f"psA1_{b}")
            mm_pass(psAB, wv_sb, vt_sb, vact_sb, bv_sb, 6, 6, DCV, f"psA2_{b}")

            # ---- build Qh[b][k, h*128+q'] = q_[k, b*128+q'] * h_mat[h, k]
            qh_sb = qhp.tile([128, JC, H_OUT * NQ], bf16, tag="qh")
            for j in range(JC):
                for h in range(H_OUT):
                    nc.vector.tensor_scalar_mul(
                        qh_sb[:, j, h * NQ:(h + 1) * NQ],
                        qact_sb[:, j, b * NQ:(b + 1) * NQ],
                        hm_sb[:, j, h:h + 1],
                    )

            # ---- stage C: logits[b,h] = (qh_h).T @ v_  -> psum [128q, 512v]
            for h in range(H_OUT):
                po = psC.tile([128, NV], f32, tag="psC")
                for j in range(JC):
                    nc.tensor.matmul(
                        po,
                        lhsT=qh_sb[:, j, h * NQ:(h + 1) * NQ],
                        rhs=vact_sb[:, j, :],
                        start=(j == 0),
                        stop=(j == JC - 1),
                    )
                o_sb = outp.tile([128, NV], f32, tag="osb")
                last = (b == BPC - 1 and h == H_OUT - 1)
                if not last:
                    nc.scalar.activation(
                        out=o_sb,
                        in_=po,
                        func=mybir.ActivationFunctionType.Identity,
                        bias=hb_sb[:, h:h + 1],
                        scale=1.0,
                    )
                    eng = nc.gpsimd if h % 2 == 0 else nc.sync
                    eng.dma_start(out=out[b, h, :, :], in_=o_sb)
                else:
                    # split the very last store so its activation, issue and
                    # transfer pipeline instead of serializing after the
                    # final matmul
                    for half, eng in ((0, nc.gpsimd), (1, nc.sync)):
                        sl = slice(half * 256, (half + 1) * 256)
                        nc.scalar.activation(
                            out=o_sb[:, sl],
                            in_=po[:, sl],
                            func=mybir.ActivationFunctionType.Identity,
                            bias=hb_sb[:, h:h + 1],
                            scale=1.0,
                        )
                        eng.dma_start(out=out[b, h, :, sl], in_=o_sb[:, sl])

    nc.compile()
    return nc


def kernel(v, q, Wv, bv, Wq, bq, h_mat, h_bias):
    import ml_dtypes
    from concourse import bass_utils

    bf16 = ml_dtypes.bfloat16

    if "nc" not in _CACHE:
        _CACHE["nc"] = _build_nc()
    nc = _CACHE["nc"]

    v = np.asarray(v, dtype=np.float32)
    q = np.asarray(q, dtype=np.float32)
    Wv = np.asarray(Wv, dtype=np.float32)
    Wq = np.asarray(Wq, dtype=np.float32)
    bv = np.asarray(bv, dtype=np.float32)
    bq = np.asarray(bq, dtype=np.float32)
    h_mat = np.asarray(h_mat, dtype=np.float32)
    h_bias = np.asarray(h_bias, dtype=np.float32)

    vT = np.ascontiguousarray(v.transpose(0, 2, 1)).astype(bf16)      # (B, 2048, 512)
    WvT = np.ascontiguousarray(Wv.T).astype(bf16)                     # (2048, 1536)
    WqT = np.ascontiguousarray(Wq.T).astype(bf16)                     # (1024, 1536)
    bvT = np.ascontiguousarray(bv.reshape(JC, 128).T)                 # (128, 12)
    bqT = np.ascontiguousarray(bq.reshape(JC, 128).T)
    # hm[p, jc, h] = h_mat[h, jc*128+p]
    hmP = np.ascontiguousarray(h_mat.reshape(H_OUT, JC, 128).transpose(2, 1, 0))
    hbB = np.ascontiguousarray(np.broadcast_to(h_bias[None, :], (128, H_OUT)))

    in_maps = []
    for c in range(N_CORES):
        bs = slice(BPC * c, BPC * (c + 1))
        qTc = np.ascontiguousarray(
            q[bs].transpose(2, 0, 1).reshape(Q_DIM, BPC * NQ)
        ).astype(bf16)
        in_maps.append({
            "vT": vT[bs],
            "qT": qTc,
            "WvT": WvT,
            "WqT": WqT,
            "bvT": bvT,
            "bqT": bqT,
            "hm": hmP,
            "hb": hbB,
        })

    res = bass_utils.run_bass_kernel_spmd(nc, in_maps, list(range(N_CORES)))
    outs = np.concatenate([res.results[c]["out"] for c in range(N_CORES)], axis=0)
    # (32, 8, 128, 512) -> (32, 8, 512, 128)
    logits = outs.transpose(0, 1, 3, 2)
    return np.ascontiguousarray(logits)
